# revision 27
# baseline (speedup 1.0000x reference)
"""Trainium2 Bass kernel for nn_Attention_55293408968939.

Full-input contract: kernel(**inputs) takes the unsharded inputs and returns
the full [1, 2048, 2048] output. Internally: 16 heads are sharded 2-per-core
across 8 NeuronCores (tensor parallel); each core computes QKV projection for
its heads, RMSNorm+3D-RoPE, non-causal attention, and its partial output
projection; the host sums the 8 partials and adds the (folded) bias row.

Per-core dataflow (all matmuls fp32r = 11-bit-mantissa RNE, fp32 accumulate):
  phase 1: qT/kT computed transposed [head_dim, tok] straight from the matmul
           (lhsT = w chunk, rhs = xT chunk); v computed natural [tok, head_dim]
           (lhsT = xT chunk, rhs = wvT chunk). RMS factor r = exp(-0.5*ln(mean
           sq + eps)) via ones-matmul partition reduction + ACT Ln/Exp; RoPE
           applied in the transposed layout with host-folded cos/sin tables
           (norm weight + pair signs folded in) using a quadrant-local
           de-interleave so the pair swap is a stream_shuffle (+-16 in each
           32-partition quadrant). attention scale and r are applied to q/k
           via a GPSIMD partition broadcast + DVE multiply.
  phase 2: per (head, 512-token q chunk): ST[k,q] = kT.T-tile @ qT (16 k
           tiles), E = exp(ST) on ACT (no max subtraction needed: scores are
           ~N(0,1)), softmax sums via ones-matmul accumulation, PV via
           lhsT = v tile accumulation -> ctxT [d, q]; normalize by a DVE
           Newton-Raphson reciprocal of the sums, GPSIMD-broadcast.
  phase 3: partial = ctxT.T @ proj_wT slice, drained and DMA'd out.

Host folds: qkv v-bias contributes exactly bias_v @ proj_w.T to the output
(softmax rows sum to 1), so it is added host-side with proj_b.
"""
import sys

sys.path.insert(0, "/opt/trn_rl_repo")

import numpy as np

NUM_HEADS = 16
N_CORES = 8
D = 128           # head dim
N = 2048          # tokens
C = 2048          # model dim
EPS = 1e-6
ROPE_THETA = 10000.0

_CACHE = {}


def _round_f32r(a):
    """Round-to-nearest-even-ish to 11 mantissa bits (fp32r) so DRAM holds
    pre-rounded values for fp32r matmul consumers."""
    u = np.ascontiguousarray(a, dtype=np.float32).view(np.uint32).astype(np.uint64)
    r = ((u + np.uint64(0x800)) & np.uint64(0xFFFFF000)).astype(np.uint32)
    return r.view(np.float32)


def _perm_quadrant():
    """Partition permutation: quadrant b lanes 0-15 = even dims of [32b,32b+32),
    lanes 16-31 = odd dims. perm[p] = original head-dim index stored at lane p."""
    perm = np.empty(128, np.int64)
    for b in range(4):
        for j in range(16):
            perm[32 * b + j] = 32 * b + 2 * j
            perm[32 * b + 16 + j] = 32 * b + 2 * j + 1
    return perm


def _rope_tables(T, H, W, head_dim):
    dh = 2 * ((head_dim // 3) // 2)
    dw = dh
    dt = head_dim - dh - dw

    def axis_ang(L, d):
        inv = 1.0 / (ROPE_THETA ** (np.arange(0, d, 2, dtype=np.float32) / d))
        return np.arange(L, dtype=np.float32)[:, None] * inv[None, :]

    at = axis_ang(T, dt)
    ah = axis_ang(H, dh)
    aw = axis_ang(W, dw)
    at_g = np.broadcast_to(at[:, None, None, :], (T, H, W, dt // 2))
    ah_g = np.broadcast_to(ah[None, :, None, :], (T, H, W, dh // 2))
    aw_g = np.broadcast_to(aw[None, None, :, :], (T, H, W, dw // 2))
    ang = np.concatenate([at_g, ah_g, aw_g], axis=-1).reshape(T * H * W, head_dim // 2)
    return np.cos(ang), np.sin(ang)  # [N, 64] fp32


def _folded_tables(cos, sin, w, perm):
    """cosT/sinT [128, N] in the quadrant-deinterleaved transposed layout with
    norm weight and rotation signs folded in.

    lane p holds dim d = perm[p], pair index i = d // 2.
    m1 coeff at lane p = cos_i * w[d].
    After the +-16 quadrant shuffle, lane p holds the partner dim value, so
    m2 coeff = -sin_i * w[d+1] for even d, +sin_i * w[d-1] for odd d."""
    n = cos.shape[0]
    cosT = np.empty((128, n), np.float32)
    sinT = np.empty((128, n), np.float32)
    for p in range(128):
        d = int(perm[p])
        i = d // 2
        cosT[p] = cos[:, i] * w[d]
        if d % 2 == 0:
            sinT[p] = -sin[:, i] * w[d + 1]
        else:
            sinT[p] = sin[:, i] * w[d - 1]
    return cosT, sinT


def _build_nc(debug=False):
    import concourse.bacc as bacc
    import concourse.mybir as mybir
    import concourse.tile as tile

    F32 = mybir.dt.float32
    F32R = mybir.dt.float32r
    AF = mybir.ActivationFunctionType
    SHUF_MASK = list(range(16, 32)) + list(range(0, 16))

    # Restrict ACT table-set choice to natural_log_exp_and_others (covers
    # Identity/Copy/Ln/Exp) so the whole kernel needs ONE table load instead
    # of alternating set loads (~1.3us each). Names/positions preserved so
    # act_func_set_id indices stay valid.
    _orig_tables = bacc.get_activation_tables

    def _one_set(arch):
        tabs = _orig_tables(arch)
        return {nm: (s if nm == "natural_log_exp_and_others" else set())
                for nm, s in tabs.items()}

    bacc.get_activation_tables = _one_set

    nc = bacc.Bacc("TRN2", target_bir_lowering=False, debug=False,
                   num_devices=N_CORES)

    # ---- DRAM I/O ----
    xT_d = nc.dram_tensor("xT", [C, N], F32R, kind="ExternalInput")
    wqk_d = nc.dram_tensor("wqkT", [C, 512], F32R, kind="ExternalInput")
    wv_d = nc.dram_tensor("wvT", [C, 256], F32R, kind="ExternalInput")
    pw_d = nc.dram_tensor("projwT", [256, C], F32R, kind="ExternalInput")
    bqk_d = nc.dram_tensor("bias_qk", [128, 4], F32, kind="ExternalInput")
    cq_d = nc.dram_tensor("cosq", [128, N], F32, kind="ExternalInput")
    sq_d = nc.dram_tensor("sinq", [128, N], F32, kind="ExternalInput")
    ck_d = nc.dram_tensor("cosk", [128, N], F32, kind="ExternalInput")
    sk_d = nc.dram_tensor("sink", [128, N], F32, kind="ExternalInput")
    ones_d = nc.dram_tensor("ones", [128, 1], F32R, kind="ExternalInput")
    eps_d = nc.dram_tensor("epsc", [1, 1], F32, kind="ExternalInput")
    out_d = nc.dram_tensor("partial", [N, C], F32, kind="ExternalOutput")
    if debug:
        dbg_qk = [nc.dram_tensor(f"dbg_qk{i}", [128, N], F32, kind="ExternalOutput")
                  for i in range(4)]
        dbg_v = nc.dram_tensor("dbg_v", [128, 16, 256], F32, kind="ExternalOutput")
        dbg_ctx = nc.dram_tensor("dbg_ctx", [128, 2, N], F32, kind="ExternalOutput")

    with tile.TileContext(nc) as tc:
        with (
            tc.tile_pool(name="persist", bufs=1) as pp,
            tc.tile_pool(name="rows", bufs=4) as rows,
            tc.tile_pool(name="tabp", bufs=1) as tabp,
        ):
            # resident SBUF tensors (per-chunk DMAs so compute can start
            # as soon as the first chunks land)
            wqk_sb = pp.tile([128, 16, 512], F32R, name="wqk_sb")
            wv_sb = pp.tile([128, 16, 256], F32R, name="wv_sb")
            pw_sb = pp.tile([128, 2, C], F32R, name="pw_sb")
            tab_dram = {"cq": cq_d, "sq": sq_d, "ck": ck_d, "sk": sk_d}
            bqk_sb = pp.tile([128, 4], F32, name="bqk_sb")
            nc.sync.dma_start(bqk_sb[:], bqk_d[:])
            ones_sb = pp.tile([128, 1], F32R, name="ones_sb")
            nc.sync.dma_start(ones_sb[:], ones_d[:])
            eps_sb = pp.tile([1, 1], F32, name="eps_sb")
            nc.sync.dma_start(eps_sb[:], eps_d[:])

            # final q/k (transposed, rope'd, scaled) and v, ctx
            qk_f = [pp.tile([128, N], F32R, name=f"qkf{i}") for i in range(4)]
            v_sb = pp.tile([128, 16, 256], F32R, name="v_sb")
            ctx_sb = pp.tile([128, 2, N], F32R, name="ctx_sb")

            # table per tensor index: 0:q0 1:k0 2:q1 3:k1
            tab_of = [("cq", "sq"), ("ck", "sk"), ("cq", "sq"), ("ck", "sk")]

            # ---------------- phase 1: QKV + RMS + RoPE ----------------
            with (
                tc.tile_pool(name="xt", bufs=4) as xtp,
                tc.tile_pool(name="qraw", bufs=6) as qrawp,
                tc.tile_pool(name="scr", bufs=2) as scr,
                tc.tile_pool(name="rbc", bufs=5) as rbcp,
                tc.tile_pool(name="ps_qk", bufs=4, space="PSUM") as ps_qk,
                tc.tile_pool(name="ps_v", bufs=1, space="PSUM") as ps_v,
                tc.tile_pool(name="ps_sq", bufs=2, space="PSUM") as ps_sq,
            ):
                def rope_stage(c4):
                    tsl = slice(c4 * 512, (c4 + 1) * 512)
                    tabt = {}
                    for nm in ("cq", "sq", "ck", "sk"):
                        tabt[nm] = tabp.tile([128, 512], F32, tag=nm,
                                             name=f"tab{nm}_{c4}")
                        nc.sync.dma_start(tabt[nm][:], tab_dram[nm][:, tsl])
                    # pass A: RMS factors (keeps the ssq matmuls unblocked on
                    # the PE FIFO after only the 4 cheap sq multiplies)
                    rbcs = {}
                    for f in (1, 3, 0, 2):   # k tensors first
                        qraw = qraw_tiles[(c4, f)]
                        sq = scr.tile([128, 512], F32R, tag="sq", name=f"sq{c4}_{f}")
                        nc.vector.tensor_mul(sq[:], qraw[:], qraw[:])
                        ssq = ps_sq.tile([1, 512], F32, tag="ssq", name=f"ssq{c4}_{f}")
                        nc.tensor.matmul(ssq[:], ones_sb[:], sq[:], start=True,
                                         stop=True)
                        lnr = rows.tile([1, 512], F32, tag="row", name=f"lnr{c4}_{f}")
                        nc.scalar.activation(lnr[:], ssq[:], AF.Ln,
                                             scale=1.0 / 128.0, bias=eps_sb[0:1, 0:1])
                        rrow = rows.tile([1, 512], F32, tag="row", name=f"rrow{c4}_{f}")
                        # r = mean_sq^-1/2 * D^-1/4  (D^-1/2 split across q and k)
                        nc.scalar.activation(rrow[:], lnr[:], AF.Exp, scale=-0.5,
                                             bias=_log_quarter(nc, pp))
                        rbc = rbcp.tile([128, 512], F32, tag="rbc", name=f"rbc{c4}_{f}")
                        nc.gpsimd.partition_broadcast(rbc[:], rrow[:])
                        rbcs[f] = rbc
                    # pass B: rotation + scaling
                    for f in (1, 3, 0, 2):
                        qraw = qraw_tiles[(c4, f)]
                        cosT = tabt[tab_of[f][0]]
                        sinT = tabt[tab_of[f][1]]
                        m1 = scr.tile([128, 512], F32, tag="m1", name=f"m1_{c4}_{f}")
                        nc.vector.tensor_mul(m1[:], qraw[:], cosT[:])
                        sh = scr.tile([128, 512], F32, tag="sh", name=f"sh{c4}_{f}")
                        nc.vector.stream_shuffle(sh[:], qraw[:], SHUF_MASK)
                        nc.vector.tensor_mul(sh[:], sh[:], sinT[:])
                        nc.vector.tensor_add(m1[:], m1[:], sh[:])
                        nc.vector.tensor_mul(qk_f[f][:, tsl], m1[:], rbcs[f][:])

                qraw_tiles = {}
                for c4 in range(4):
                    tsl = slice(c4 * 512, (c4 + 1) * 512)
                    qk_ps = [ps_qk.tile([128, 512], F32, tag="qkps", name=f"qkps{c4}_{_f}") for _f in range(4)]
                    # [128,1024] = 2 banks, two 256-wide v regions per bank.
                    # Only the first region per bank passes start=True (clears
                    # the whole bank); the second region's first matmul relies
                    # on the cleared has_written bits to overwrite, which is
                    # safe because the PE executes matmuls strictly in program
                    # order.
                    v_ps = ps_v.tile([128, 1024], F32, tag="vps", name=f"vps{c4}")
                    for i in range(16):
                        if c4 == 0:
                            nc.sync.dma_start(wqk_sb[:, i, :],
                                              wqk_d[i * 128:(i + 1) * 128, :])
                            nc.sync.dma_start(wv_sb[:, i, :],
                                              wv_d[i * 128:(i + 1) * 128, :])
                        xt = xtp.tile([128, 512], F32R, tag="xt", name=f"xt{c4}_{i}")
                        nc.sync.dma_start(xt[:], xT_d[i * 128:(i + 1) * 128, tsl])
                        for f in range(4):
                            nc.tensor.matmul(qk_ps[f][:],
                                             wqk_sb[:, i, f * 128:(f + 1) * 128],
                                             xt[:], start=(i == 0), stop=(i == 15))
                        for j in range(4):
                            nc.tensor.matmul(v_ps[:, j * 256:(j + 1) * 256],
                                             xt[:, j * 128:(j + 1) * 128],
                                             wv_sb[:, i, :],
                                             start=(i == 0 and j % 2 == 0),
                                             stop=(i == 15),
                                             skip_group_check=True)
                    # drain v: [tok 128, 256] tiles -> v_sb[:, kt, :]
                    for j in range(4):
                        kt = c4 * 4 + j
                        nc.vector.tensor_copy(v_sb[:, kt, :],
                                              v_ps[:, j * 256:(j + 1) * 256])
                    # drain q/k with bias; rope for the PREVIOUS chunk (keeps
                    # the PE FIFO free of ops that wait on the ACT/DVE chain)
                    for f in range(4):
                        qraw = qrawp.tile([128, 512], F32, tag="qraw", name=f"qraw{c4}_{f}")
                        nc.scalar.activation(qraw[:], qk_ps[f][:], AF.Identity,
                                             bias=bqk_sb[:, f:f + 1], scale=1.0)
                        qraw_tiles[(c4, f)] = qraw
                    if c4 > 0:
                        rope_stage(c4 - 1)
                rope_stage(3)

            for fc in range(2):
                nc.sync.dma_start(pw_sb[:, fc, :],
                                  pw_d[fc * 128:(fc + 1) * 128, :])

            # ------------- phase 2+3: attention + fused projection -------------
            with (
                tc.tile_pool(name="ep", bufs=3) as ep,
                tc.tile_pool(name="invb", bufs=2) as invbp,
                tc.tile_pool(name="outp", bufs=4) as outp,
                tc.tile_pool(name="ps_st", bufs=3, space="PSUM") as ps_st,
                tc.tile_pool(name="ps_ctx", bufs=2, space="PSUM") as ps_ctx,
                tc.tile_pool(name="ps_ssum", bufs=1, space="PSUM") as ps_ssum,
                tc.tile_pool(name="ps_o", bufs=2, space="PSUM") as ps_o,
            ):
                def proj_stage(qc):
                    for mt in range(4 * qc, 4 * qc + 4):
                        msl = slice(mt * 128, (mt + 1) * 128)
                        for oc in range(4):
                            osl = slice(oc * 512, (oc + 1) * 512)
                            po = ps_o.tile([128, 512], F32, tag="po", name=f"po{mt}_{oc}")
                            nc.tensor.matmul(po[:], ctx_sb[:, 0, msl], pw_sb[:, 0, osl],
                                             start=True, stop=False)
                            nc.tensor.matmul(po[:], ctx_sb[:, 1, msl], pw_sb[:, 1, osl],
                                             start=False, stop=True)
                            ot = outp.tile([128, 512], F32, tag="ot", name=f"ot{mt}_{oc}")
                            nc.vector.tensor_copy(ot[:], po[:])
                            nc.sync.dma_start(out_d[msl, osl], ot[:])

                for qc in range(4):
                    qsl = slice(qc * 512, (qc + 1) * 512)
                    for h in range(2):
                        qT = qk_f[2 * h]
                        kT = qk_f[2 * h + 1]
                        ctx_ps = ps_ctx.tile([128, 512], F32, tag="ctxps", name=f"ctxps{h}_{qc}")
                        ssum = ps_ssum.tile([1, 512], F32, tag="ssum", name=f"ssum{h}_{qc}")
                        for kt in range(16):
                            st = ps_st.tile([128, 512], F32, tag="st", name=f"st{h}_{qc}_{kt}")
                            nc.tensor.matmul(st[:], kT[:, kt * 128:(kt + 1) * 128],
                                             qT[:, qsl], start=True, stop=True)
                            e = ep.tile([128, 512], F32R, tag="e", name=f"e{h}_{qc}_{kt}")
                            nc.scalar.activation(e[:], st[:], AF.Exp)
                            nc.tensor.matmul(ssum[:], ones_sb[:], e[:],
                                             start=(kt == 0), stop=(kt == 15))
                            nc.tensor.matmul(ctx_ps[:],
                                             v_sb[:, kt, h * 128:(h + 1) * 128],
                                             e[:], start=(kt == 0), stop=(kt == 15))
                        ssc = rows.tile([1, 512], F32, tag="row", name=f"ssc{h}_{qc}")
                        nc.vector.tensor_copy(ssc[:], ssum[:])
                        scr2 = rows.tile([1, 512], F32, tag="row", name=f"scr{h}_{qc}")
                        inv = rows.tile([1, 512], F32, tag="row", name=f"inv{h}_{qc}")
                        nc.vector.reciprocal_approx_accurate(inv[:], ssc[:], scr2[:])
                        invb = invbp.tile([128, 512], F32, tag="invb", name=f"invb{h}_{qc}")
                        nc.gpsimd.partition_broadcast(invb[:], inv[:])
                        nc.vector.tensor_mul(ctx_sb[:, h, qsl], ctx_ps[:], invb[:])
                    if qc > 0:
                        proj_stage(qc - 1)
                proj_stage(3)

            if debug:
                for i in range(4):
                    nc.sync.dma_start(dbg_qk[i][:], qk_f[i][:].bitcast(F32))
                nc.sync.dma_start(dbg_v[:], v_sb[:].bitcast(F32))
                nc.sync.dma_start(dbg_ctx[:], ctx_sb[:].bitcast(F32))

    try:
        nc.compile()
    finally:
        bacc.get_activation_tables = _orig_tables
    return nc


_LOGQ = {}


def _log_quarter(nc, pp):
    """[1,1] SBUF const holding -0.25*ln(128) (attention-scale split)."""
    import concourse.mybir as mybir
    key = id(nc)
    if key not in _LOGQ:
        t = pp.tile([1, 1], mybir.dt.float32, name="logq")
        nc.vector.memset(t[:], float(-0.25 * np.log(128.0)))
        _LOGQ[key] = t
    return _LOGQ[key][0:1, 0:1]


def _host_prep(x, qkv_w, qkv_b, proj_w, proj_b, q_norm_w, k_norm_w, T, H, W):
    perm = _perm_quadrant()
    cos, sin = _rope_tables(T, H, W, D)
    cosq, sinq = _folded_tables(cos, sin, np.asarray(q_norm_w, np.float32), perm)
    cosk, sink = _folded_tables(cos, sin, np.asarray(k_norm_w, np.float32), perm)

    xT = _round_f32r(np.asarray(x, np.float32)[0].T)
    qkv_w = np.asarray(qkv_w, np.float32)
    qkv_b = np.asarray(qkv_b, np.float32)
    proj_w = np.asarray(proj_w, np.float32)

    shared = dict(xT=xT, cosq=cosq, sinq=sinq, cosk=cosk, sink=sink,
                  ones=np.ones((128, 1), np.float32),
                  epsc=np.full((1, 1), EPS, np.float32))
    in_maps = []
    for c in range(N_CORES):
        h0 = 2 * c
        wq = [qkv_w[(h0 + j) * D:(h0 + j + 1) * D][perm] for j in range(2)]
        wk = [qkv_w[C + (h0 + j) * D:C + (h0 + j + 1) * D][perm] for j in range(2)]
        bq = [qkv_b[(h0 + j) * D:(h0 + j + 1) * D][perm] for j in range(2)]
        bk = [qkv_b[C + (h0 + j) * D:C + (h0 + j + 1) * D][perm] for j in range(2)]
        wqkT = np.concatenate([wq[0], wk[0], wq[1], wk[1]], axis=0).T
        bias_qk = np.stack([bq[0], bk[0], bq[1], bk[1]], axis=1)
        wvT = qkv_w[2 * C + h0 * D:2 * C + (h0 + 2) * D].T
        projwT = proj_w[:, h0 * D:(h0 + 2) * D].T
        in_maps.append(dict(shared,
                            wqkT=_round_f32r(wqkT),
                            wvT=_round_f32r(wvT),
                            projwT=_round_f32r(projwT),
                            bias_qk=np.ascontiguousarray(bias_qk)))
    v_bias = qkv_b[2 * C:]
    bias_row = (np.asarray(proj_b, np.float32).astype(np.float64)
                + v_bias.astype(np.float64) @ proj_w.astype(np.float64).T)
    return in_maps, bias_row


def kernel(x, qkv_w, qkv_b, proj_w, proj_b, q_norm_w, k_norm_w,
           t_dim, h_dim, w_dim):
    from concourse import bass_utils

    T, H, W = int(t_dim), int(h_dim), int(w_dim)
    if "nc" not in _CACHE:
        _CACHE["nc"] = _build_nc()
    nc = _CACHE["nc"]

    in_maps, bias_row = _host_prep(x, qkv_w, qkv_b, proj_w, proj_b,
                                   q_norm_w, k_norm_w, T, H, W)
    res = bass_utils.run_bass_kernel_spmd(nc, in_maps,
                                          core_ids=list(range(N_CORES)))
    total = np.zeros((N, C), np.float64)
    for r in res.results:
        total += r["partial"]
    out = (total + bias_row[None, :]).astype(np.float32)[None]
    return out


# revision 31
# speedup vs baseline: 1.0324x; 1.0324x over previous
"""Trainium2 Bass kernel for nn_Attention_55293408968939.

Full-input contract: kernel(**inputs) takes the unsharded inputs and returns
the full [1, 2048, 2048] output. Internally: 16 heads are sharded 2-per-core
across 8 NeuronCores (tensor parallel); each core computes QKV projection for
its heads, RMSNorm+3D-RoPE, non-causal attention, and its partial output
projection; the host sums the 8 partials and adds the (folded) bias row.

Per-core dataflow (all matmuls fp32r = 11-bit-mantissa RNE, fp32 accumulate):
  phase 1: qT/kT computed transposed [head_dim, tok] straight from the matmul
           (lhsT = w chunk, rhs = xT chunk); v computed natural [tok, head_dim]
           (lhsT = xT chunk, rhs = wvT chunk). RMS factor r = exp(-0.5*ln(mean
           sq + eps)) via ones-matmul partition reduction + ACT Ln/Exp; RoPE
           applied in the transposed layout with host-folded cos/sin tables
           (norm weight + pair signs folded in) using a quadrant-local
           de-interleave so the pair swap is a stream_shuffle (+-16 in each
           32-partition quadrant). attention scale and r are applied to q/k
           via a GPSIMD partition broadcast + DVE multiply.
  phase 2: per (head, 512-token q chunk): ST[k,q] = kT.T-tile @ qT (16 k
           tiles), E = exp(ST) on ACT (no max subtraction needed: scores are
           ~N(0,1)), softmax sums via ones-matmul accumulation, PV via
           lhsT = v tile accumulation -> ctxT [d, q]; normalize by a DVE
           Newton-Raphson reciprocal of the sums, GPSIMD-broadcast.
  phase 3: partial = ctxT.T @ proj_wT slice, drained and DMA'd out.

Host folds: qkv v-bias contributes exactly bias_v @ proj_w.T to the output
(softmax rows sum to 1), so it is added host-side with proj_b.
"""
import sys

sys.path.insert(0, "/opt/trn_rl_repo")

import numpy as np

NUM_HEADS = 16
N_CORES = 8
D = 128           # head dim
N = 2048          # tokens
C = 2048          # model dim
EPS = 1e-6
ROPE_THETA = 10000.0

_CACHE = {}


def _round_f32r(a):
    """Round-to-nearest-even-ish to 11 mantissa bits (fp32r) so DRAM holds
    pre-rounded values for fp32r matmul consumers."""
    u = np.ascontiguousarray(a, dtype=np.float32).view(np.uint32).astype(np.uint64)
    r = ((u + np.uint64(0x800)) & np.uint64(0xFFFFF000)).astype(np.uint32)
    return r.view(np.float32)


def _perm_quadrant():
    """Partition permutation: quadrant b lanes 0-15 = even dims of [32b,32b+32),
    lanes 16-31 = odd dims. perm[p] = original head-dim index stored at lane p."""
    perm = np.empty(128, np.int64)
    for b in range(4):
        for j in range(16):
            perm[32 * b + j] = 32 * b + 2 * j
            perm[32 * b + 16 + j] = 32 * b + 2 * j + 1
    return perm


def _rope_tables(T, H, W, head_dim):
    dh = 2 * ((head_dim // 3) // 2)
    dw = dh
    dt = head_dim - dh - dw

    def axis_ang(L, d):
        inv = 1.0 / (ROPE_THETA ** (np.arange(0, d, 2, dtype=np.float32) / d))
        return np.arange(L, dtype=np.float32)[:, None] * inv[None, :]

    at = axis_ang(T, dt)
    ah = axis_ang(H, dh)
    aw = axis_ang(W, dw)
    at_g = np.broadcast_to(at[:, None, None, :], (T, H, W, dt // 2))
    ah_g = np.broadcast_to(ah[None, :, None, :], (T, H, W, dh // 2))
    aw_g = np.broadcast_to(aw[None, None, :, :], (T, H, W, dw // 2))
    ang = np.concatenate([at_g, ah_g, aw_g], axis=-1).reshape(T * H * W, head_dim // 2)
    return np.cos(ang), np.sin(ang)  # [N, 64] fp32


def _folded_tables(cos, sin, w, perm):
    """cosT/sinT [128, N] in the quadrant-deinterleaved transposed layout with
    norm weight and rotation signs folded in.

    lane p holds dim d = perm[p], pair index i = d // 2.
    m1 coeff at lane p = cos_i * w[d].
    After the +-16 quadrant shuffle, lane p holds the partner dim value, so
    m2 coeff = -sin_i * w[d+1] for even d, +sin_i * w[d-1] for odd d."""
    n = cos.shape[0]
    cosT = np.empty((128, n), np.float32)
    sinT = np.empty((128, n), np.float32)
    for p in range(128):
        d = int(perm[p])
        i = d // 2
        cosT[p] = cos[:, i] * w[d]
        if d % 2 == 0:
            sinT[p] = -sin[:, i] * w[d + 1]
        else:
            sinT[p] = sin[:, i] * w[d - 1]
    return cosT, sinT


def _build_nc(debug=False):
    import concourse.bacc as bacc
    import concourse.mybir as mybir
    import concourse.tile as tile

    F32 = mybir.dt.float32
    F32R = mybir.dt.float32r
    AF = mybir.ActivationFunctionType
    SHUF_MASK = list(range(16, 32)) + list(range(0, 16))

    # Restrict ACT table-set choice to natural_log_exp_and_others (covers
    # Identity/Copy/Ln/Exp) so the whole kernel needs ONE table load instead
    # of alternating set loads (~1.3us each). Names/positions preserved so
    # act_func_set_id indices stay valid.
    _orig_tables = bacc.get_activation_tables

    def _one_set(arch):
        tabs = _orig_tables(arch)
        return {nm: (s if nm == "natural_log_exp_and_others" else set())
                for nm, s in tabs.items()}

    bacc.get_activation_tables = _one_set

    nc = bacc.Bacc("TRN2", target_bir_lowering=False, debug=False,
                   num_devices=N_CORES)

    # ---- DRAM I/O ----
    xT_d = nc.dram_tensor("xT", [C, N], F32R, kind="ExternalInput")
    wqk_d = nc.dram_tensor("wqkT", [C, 512], F32R, kind="ExternalInput")
    wv_d = nc.dram_tensor("wvT", [C, 256], F32R, kind="ExternalInput")
    pw_d = nc.dram_tensor("projwT", [256, C], F32R, kind="ExternalInput")
    bqk_d = nc.dram_tensor("bias_qk", [128, 4], F32, kind="ExternalInput")
    cq_d = nc.dram_tensor("cosq", [128, N], F32, kind="ExternalInput")
    sq_d = nc.dram_tensor("sinq", [128, N], F32, kind="ExternalInput")
    ck_d = nc.dram_tensor("cosk", [128, N], F32, kind="ExternalInput")
    sk_d = nc.dram_tensor("sink", [128, N], F32, kind="ExternalInput")
    ones_d = nc.dram_tensor("ones", [128, 1], F32R, kind="ExternalInput")
    eps_d = nc.dram_tensor("epsc", [1, 1], F32, kind="ExternalInput")
    out_d = nc.dram_tensor("partial", [N, C], F32, kind="ExternalOutput")
    if debug:
        dbg_qk = [nc.dram_tensor(f"dbg_qk{i}", [128, N], F32, kind="ExternalOutput")
                  for i in range(4)]
        dbg_v = nc.dram_tensor("dbg_v", [128, 16, 256], F32, kind="ExternalOutput")
        dbg_ctx = nc.dram_tensor("dbg_ctx", [128, 2, N], F32, kind="ExternalOutput")

    with tile.TileContext(nc) as tc:
        with (
            tc.tile_pool(name="persist", bufs=1) as pp,
            tc.tile_pool(name="rows", bufs=6) as rows,
            tc.tile_pool(name="tabp", bufs=1) as tabp,
        ):
            # resident SBUF tensors (per-chunk DMAs so compute can start
            # as soon as the first chunks land)
            wqk_sb = pp.tile([128, 16, 512], F32R, name="wqk_sb")
            wv_sb = pp.tile([128, 16, 256], F32R, name="wv_sb")
            pw_sb = pp.tile([128, 2, C], F32R, name="pw_sb")
            tab_dram = {"cq": cq_d, "sq": sq_d, "ck": ck_d, "sk": sk_d}
            bqk_sb = pp.tile([128, 4], F32, name="bqk_sb")
            nc.sync.dma_start(bqk_sb[:], bqk_d[:])
            ones_sb = pp.tile([128, 1], F32R, name="ones_sb")
            nc.sync.dma_start(ones_sb[:], ones_d[:])
            eps_sb = pp.tile([1, 1], F32, name="eps_sb")
            nc.sync.dma_start(eps_sb[:], eps_d[:])

            # final q/k (transposed, rope'd, scaled) and v, ctx
            qk_f = [pp.tile([128, N], F32R, name=f"qkf{i}") for i in range(4)]
            v_sb = pp.tile([128, 16, 256], F32R, name="v_sb")
            ctx_sb = pp.tile([128, 2, N], F32R, name="ctx_sb")

            # table per tensor index: 0:q0 1:k0 2:q1 3:k1
            tab_of = [("cq", "sq"), ("ck", "sk"), ("cq", "sq"), ("ck", "sk")]

            # ---------------- phase 1: QKV + RMS + RoPE ----------------
            with (
                tc.tile_pool(name="xt", bufs=4) as xtp,
                tc.tile_pool(name="qraw", bufs=6) as qrawp,
                tc.tile_pool(name="scr", bufs=3) as scr,
                tc.tile_pool(name="rbc", bufs=5) as rbcp,
                tc.tile_pool(name="ps_qk", bufs=4, space="PSUM") as ps_qk,
                tc.tile_pool(name="ps_v", bufs=1, space="PSUM") as ps_v,
                tc.tile_pool(name="ps_sq", bufs=2, space="PSUM") as ps_sq,
            ):
                def rope_stage(c4):
                    tsl = slice(c4 * 512, (c4 + 1) * 512)
                    tabt = {}
                    for nm in ("cq", "sq", "ck", "sk"):
                        tabt[nm] = tabp.tile([128, 512], F32, tag=nm,
                                             name=f"tab{nm}_{c4}")
                        nc.sync.dma_start(tabt[nm][:], tab_dram[nm][:, tsl])
                    # pass A: RMS factors (keeps the ssq matmuls unblocked on
                    # the PE FIFO after only the 4 cheap sq multiplies)
                    rbcs = {}
                    for f in (1, 3, 0, 2):   # k tensors first
                        qraw = qraw_tiles[(c4, f)]
                        sq = scr.tile([128, 512], F32R, tag="sq", name=f"sq{c4}_{f}")
                        nc.vector.tensor_mul(sq[:], qraw[:], qraw[:])
                        ssq = ps_sq.tile([1, 512], F32, tag="ssq", name=f"ssq{c4}_{f}")
                        nc.tensor.matmul(ssq[:], ones_sb[:], sq[:], start=True,
                                         stop=True)
                        lnr = rows.tile([1, 512], F32, tag="row", name=f"lnr{c4}_{f}")
                        nc.scalar.activation(lnr[:], ssq[:], AF.Ln,
                                             scale=1.0 / 128.0, bias=eps_sb[0:1, 0:1])
                        rrow = rows.tile([1, 512], F32, tag="row", name=f"rrow{c4}_{f}")
                        # r = mean_sq^-1/2 * D^-1/4  (D^-1/2 split across q and k)
                        nc.scalar.activation(rrow[:], lnr[:], AF.Exp, scale=-0.5,
                                             bias=_log_quarter(nc, pp))
                        rbc = rbcp.tile([128, 512], F32, tag="rbc", name=f"rbc{c4}_{f}")
                        nc.gpsimd.partition_broadcast(rbc[:], rrow[:])
                        rbcs[f] = rbc
                    # pass B: rotation + scaling
                    for f in (1, 3, 0, 2):
                        qraw = qraw_tiles[(c4, f)]
                        cosT = tabt[tab_of[f][0]]
                        sinT = tabt[tab_of[f][1]]
                        m1 = scr.tile([128, 512], F32, tag="m1", name=f"m1_{c4}_{f}")
                        nc.vector.tensor_mul(m1[:], qraw[:], cosT[:])
                        sh = scr.tile([128, 512], F32, tag="sh", name=f"sh{c4}_{f}")
                        nc.vector.stream_shuffle(sh[:], qraw[:], SHUF_MASK)
                        nc.vector.tensor_mul(sh[:], sh[:], sinT[:])
                        nc.vector.tensor_add(m1[:], m1[:], sh[:])
                        nc.vector.tensor_mul(qk_f[f][:, tsl], m1[:], rbcs[f][:])

                qraw_tiles = {}
                for c4 in range(4):
                    tsl = slice(c4 * 512, (c4 + 1) * 512)
                    qk_ps = [ps_qk.tile([128, 512], F32, tag="qkps", name=f"qkps{c4}_{_f}") for _f in range(4)]
                    # [128,1024] = 2 banks, two 256-wide v regions per bank.
                    # Only the first region per bank passes start=True (clears
                    # the whole bank); the second region's first matmul relies
                    # on the cleared has_written bits to overwrite, which is
                    # safe because the PE executes matmuls strictly in program
                    # order.
                    v_ps = ps_v.tile([128, 1024], F32, tag="vps", name=f"vps{c4}")
                    for i in range(16):
                        if c4 == 0:
                            nc.sync.dma_start(wqk_sb[:, i, :],
                                              wqk_d[i * 128:(i + 1) * 128, :])
                            nc.sync.dma_start(wv_sb[:, i, :],
                                              wv_d[i * 128:(i + 1) * 128, :])
                        xt = xtp.tile([128, 512], F32R, tag="xt", name=f"xt{c4}_{i}")
                        nc.sync.dma_start(xt[:], xT_d[i * 128:(i + 1) * 128, tsl])
                        for f in range(4):
                            nc.tensor.matmul(qk_ps[f][:],
                                             wqk_sb[:, i, f * 128:(f + 1) * 128],
                                             xt[:], start=(i == 0), stop=(i == 15))
                        for j in range(4):
                            nc.tensor.matmul(v_ps[:, j * 256:(j + 1) * 256],
                                             xt[:, j * 128:(j + 1) * 128],
                                             wv_sb[:, i, :],
                                             start=(i == 0 and j % 2 == 0),
                                             stop=(i == 15),
                                             skip_group_check=True)
                    # drain v: [tok 128, 256] tiles -> v_sb[:, kt, :]
                    for j in range(4):
                        kt = c4 * 4 + j
                        nc.vector.tensor_copy(v_sb[:, kt, :],
                                              v_ps[:, j * 256:(j + 1) * 256])
                    # drain q/k with bias; rope for the PREVIOUS chunk (keeps
                    # the PE FIFO free of ops that wait on the ACT/DVE chain)
                    for f in range(4):
                        qraw = qrawp.tile([128, 512], F32, tag="qraw", name=f"qraw{c4}_{f}")
                        nc.scalar.activation(qraw[:], qk_ps[f][:], AF.Identity,
                                             bias=bqk_sb[:, f:f + 1], scale=1.0)
                        qraw_tiles[(c4, f)] = qraw
                    if c4 > 0:
                        rope_stage(c4 - 1)
                rope_stage(3)

            for fc in range(2):
                nc.sync.dma_start(pw_sb[:, fc, :],
                                  pw_d[fc * 128:(fc + 1) * 128, :])

            # ------------- phase 2+3: attention + fused projection -------------
            with (
                tc.tile_pool(name="ep", bufs=4) as ep,
                tc.tile_pool(name="invb", bufs=3) as invbp,
                tc.tile_pool(name="outp", bufs=8) as outp,
                tc.tile_pool(name="ps_st", bufs=3, space="PSUM") as ps_st,
                tc.tile_pool(name="ps_ctx", bufs=2, space="PSUM") as ps_ctx,
                tc.tile_pool(name="ps_ssum", bufs=1, space="PSUM") as ps_ssum,
                tc.tile_pool(name="ps_o", bufs=2, space="PSUM") as ps_o,
            ):
                def proj_stage(qc, last=False):
                    for mt in range(4 * qc, 4 * qc + 4):
                        msl = slice(mt * 128, (mt + 1) * 128)
                        for oc in range(4):
                            osl = slice(oc * 512, (oc + 1) * 512)
                            po = ps_o.tile([128, 512], F32, tag="po", name=f"po{mt}_{oc}")
                            nc.tensor.matmul(po[:], ctx_sb[:, 0, msl], pw_sb[:, 0, osl],
                                             start=True, stop=False)
                            nc.tensor.matmul(po[:], ctx_sb[:, 1, msl], pw_sb[:, 1, osl],
                                             start=False, stop=True)
                            ot = outp.tile([128, 512], F32, tag="ot", name=f"ot{mt}_{oc}")
                            if last and oc % 2 == 1:
                                nc.scalar.copy(ot[:], po[:])
                            else:
                                nc.vector.tensor_copy(ot[:], po[:])
                            nc.sync.dma_start(out_d[msl, osl], ot[:])

                for qc in range(4):
                    qsl = slice(qc * 512, (qc + 1) * 512)
                    for h in range(2):
                        qT = qk_f[2 * h]
                        kT = qk_f[2 * h + 1]
                        ctx_ps = ps_ctx.tile([128, 512], F32, tag="ctxps", name=f"ctxps{h}_{qc}")
                        ssum = ps_ssum.tile([1, 512], F32, tag="ssum", name=f"ssum{h}_{qc}")
                        for kt in range(16):
                            st = ps_st.tile([128, 512], F32, tag="st", name=f"st{h}_{qc}_{kt}")
                            nc.tensor.matmul(st[:], kT[:, kt * 128:(kt + 1) * 128],
                                             qT[:, qsl], start=True, stop=True)
                            e = ep.tile([128, 512], F32R, tag="e", name=f"e{h}_{qc}_{kt}")
                            nc.scalar.activation(e[:], st[:], AF.Exp)
                            nc.tensor.matmul(ssum[:], ones_sb[:], e[:],
                                             start=(kt == 0), stop=(kt == 15))
                            nc.tensor.matmul(ctx_ps[:],
                                             v_sb[:, kt, h * 128:(h + 1) * 128],
                                             e[:], start=(kt == 0), stop=(kt == 15))
                        ssc = rows.tile([1, 512], F32, tag="row", name=f"ssc{h}_{qc}")
                        nc.vector.tensor_copy(ssc[:], ssum[:])
                        scr2 = rows.tile([1, 512], F32, tag="row", name=f"scr{h}_{qc}")
                        inv = rows.tile([1, 512], F32, tag="row", name=f"inv{h}_{qc}")
                        nc.vector.reciprocal_approx_accurate(inv[:], ssc[:], scr2[:])
                        invb = invbp.tile([128, 512], F32, tag="invb", name=f"invb{h}_{qc}")
                        nc.gpsimd.partition_broadcast(invb[:], inv[:])
                        nc.vector.tensor_mul(ctx_sb[:, h, qsl], ctx_ps[:], invb[:])
                    if qc > 0:
                        proj_stage(qc - 1)
                proj_stage(3, last=True)

            if debug:
                for i in range(4):
                    nc.sync.dma_start(dbg_qk[i][:], qk_f[i][:].bitcast(F32))
                nc.sync.dma_start(dbg_v[:], v_sb[:].bitcast(F32))
                nc.sync.dma_start(dbg_ctx[:], ctx_sb[:].bitcast(F32))

    try:
        nc.compile()
    finally:
        bacc.get_activation_tables = _orig_tables
    return nc


_LOGQ = {}


def _log_quarter(nc, pp):
    """[1,1] SBUF const holding -0.25*ln(128) (attention-scale split)."""
    import concourse.mybir as mybir
    key = id(nc)
    if key not in _LOGQ:
        t = pp.tile([1, 1], mybir.dt.float32, name="logq")
        nc.vector.memset(t[:], float(-0.25 * np.log(128.0)))
        _LOGQ[key] = t
    return _LOGQ[key][0:1, 0:1]


def _host_prep(x, qkv_w, qkv_b, proj_w, proj_b, q_norm_w, k_norm_w, T, H, W):
    perm = _perm_quadrant()
    cos, sin = _rope_tables(T, H, W, D)
    cosq, sinq = _folded_tables(cos, sin, np.asarray(q_norm_w, np.float32), perm)
    cosk, sink = _folded_tables(cos, sin, np.asarray(k_norm_w, np.float32), perm)

    xT = _round_f32r(np.asarray(x, np.float32)[0].T)
    qkv_w = np.asarray(qkv_w, np.float32)
    qkv_b = np.asarray(qkv_b, np.float32)
    proj_w = np.asarray(proj_w, np.float32)

    shared = dict(xT=xT, cosq=cosq, sinq=sinq, cosk=cosk, sink=sink,
                  ones=np.ones((128, 1), np.float32),
                  epsc=np.full((1, 1), EPS, np.float32))
    in_maps = []
    for c in range(N_CORES):
        h0 = 2 * c
        wq = [qkv_w[(h0 + j) * D:(h0 + j + 1) * D][perm] for j in range(2)]
        wk = [qkv_w[C + (h0 + j) * D:C + (h0 + j + 1) * D][perm] for j in range(2)]
        bq = [qkv_b[(h0 + j) * D:(h0 + j + 1) * D][perm] for j in range(2)]
        bk = [qkv_b[C + (h0 + j) * D:C + (h0 + j + 1) * D][perm] for j in range(2)]
        wqkT = np.concatenate([wq[0], wk[0], wq[1], wk[1]], axis=0).T
        bias_qk = np.stack([bq[0], bk[0], bq[1], bk[1]], axis=1)
        wvT = qkv_w[2 * C + h0 * D:2 * C + (h0 + 2) * D].T
        projwT = proj_w[:, h0 * D:(h0 + 2) * D].T
        in_maps.append(dict(shared,
                            wqkT=_round_f32r(wqkT),
                            wvT=_round_f32r(wvT),
                            projwT=_round_f32r(projwT),
                            bias_qk=np.ascontiguousarray(bias_qk)))
    v_bias = qkv_b[2 * C:]
    bias_row = (np.asarray(proj_b, np.float32).astype(np.float64)
                + v_bias.astype(np.float64) @ proj_w.astype(np.float64).T)
    return in_maps, bias_row


def kernel(x, qkv_w, qkv_b, proj_w, proj_b, q_norm_w, k_norm_w,
           t_dim, h_dim, w_dim):
    from concourse import bass_utils

    T, H, W = int(t_dim), int(h_dim), int(w_dim)
    if "nc" not in _CACHE:
        _CACHE["nc"] = _build_nc()
    nc = _CACHE["nc"]

    in_maps, bias_row = _host_prep(x, qkv_w, qkv_b, proj_w, proj_b,
                                   q_norm_w, k_norm_w, T, H, W)
    res = bass_utils.run_bass_kernel_spmd(nc, in_maps,
                                          core_ids=list(range(N_CORES)))
    total = np.zeros((N, C), np.float64)
    for r in res.results:
        total += r["partial"]
    out = (total + bias_row[None, :]).astype(np.float32)[None]
    return out


# revision 38
# speedup vs baseline: 1.0475x; 1.0146x over previous
"""Trainium2 Bass kernel for nn_Attention_55293408968939.

Full-input contract: kernel(**inputs) takes the unsharded inputs and returns
the full [1, 2048, 2048] output. Internally: 16 heads are sharded 2-per-core
across 8 NeuronCores (tensor parallel); each core computes QKV projection for
its heads, RMSNorm+3D-RoPE, non-causal attention, and its partial output
projection; the host sums the 8 partials and adds the (folded) bias row.

Per-core dataflow (all matmuls fp32r = 11-bit-mantissa RNE, fp32 accumulate):
  phase 1: qT/kT computed transposed [head_dim, tok] straight from the matmul
           (lhsT = w chunk, rhs = xT chunk); v computed natural [tok, head_dim]
           (lhsT = xT chunk, rhs = wvT chunk). RMS factor r = exp(-0.5*ln(mean
           sq + eps)) via ones-matmul partition reduction + ACT Ln/Exp; RoPE
           applied in the transposed layout with host-folded cos/sin tables
           (norm weight + pair signs folded in) using a quadrant-local
           de-interleave so the pair swap is a stream_shuffle (+-16 in each
           32-partition quadrant). attention scale and r are applied to q/k
           via a GPSIMD partition broadcast + DVE multiply.
  phase 2: per (head, 512-token q chunk): ST[k,q] = kT.T-tile @ qT (16 k
           tiles), E = exp(ST) on ACT (no max subtraction needed: scores are
           ~N(0,1)), softmax sums via ones-matmul accumulation, PV via
           lhsT = v tile accumulation -> ctxT [d, q]; normalize by a DVE
           Newton-Raphson reciprocal of the sums, GPSIMD-broadcast.
  phase 3: partial = ctxT.T @ proj_wT slice, drained and DMA'd out.

Host folds: qkv v-bias contributes exactly bias_v @ proj_w.T to the output
(softmax rows sum to 1), so it is added host-side with proj_b.
"""
import sys

sys.path.insert(0, "/opt/trn_rl_repo")

import numpy as np

NUM_HEADS = 16
N_CORES = 8
D = 128           # head dim
N = 2048          # tokens
C = 2048          # model dim
EPS = 1e-6
ROPE_THETA = 10000.0

_CACHE = {}


def _round_f32r(a):
    """Round-to-nearest-even-ish to 11 mantissa bits (fp32r) so DRAM holds
    pre-rounded values for fp32r matmul consumers."""
    u = np.ascontiguousarray(a, dtype=np.float32).view(np.uint32).astype(np.uint64)
    r = ((u + np.uint64(0x800)) & np.uint64(0xFFFFF000)).astype(np.uint32)
    return r.view(np.float32)


def _perm_quadrant():
    """Partition permutation: quadrant b lanes 0-15 = even dims of [32b,32b+32),
    lanes 16-31 = odd dims. perm[p] = original head-dim index stored at lane p."""
    perm = np.empty(128, np.int64)
    for b in range(4):
        for j in range(16):
            perm[32 * b + j] = 32 * b + 2 * j
            perm[32 * b + 16 + j] = 32 * b + 2 * j + 1
    return perm


def _rope_tables(T, H, W, head_dim):
    dh = 2 * ((head_dim // 3) // 2)
    dw = dh
    dt = head_dim - dh - dw

    def axis_ang(L, d):
        inv = 1.0 / (ROPE_THETA ** (np.arange(0, d, 2, dtype=np.float32) / d))
        return np.arange(L, dtype=np.float32)[:, None] * inv[None, :]

    at = axis_ang(T, dt)
    ah = axis_ang(H, dh)
    aw = axis_ang(W, dw)
    at_g = np.broadcast_to(at[:, None, None, :], (T, H, W, dt // 2))
    ah_g = np.broadcast_to(ah[None, :, None, :], (T, H, W, dh // 2))
    aw_g = np.broadcast_to(aw[None, None, :, :], (T, H, W, dw // 2))
    ang = np.concatenate([at_g, ah_g, aw_g], axis=-1).reshape(T * H * W, head_dim // 2)
    return np.cos(ang), np.sin(ang)  # [N, 64] fp32


def _folded_tables(cos, sin, w, perm):
    """cosT/sinT [128, N] in the quadrant-deinterleaved transposed layout with
    norm weight and rotation signs folded in.

    lane p holds dim d = perm[p], pair index i = d // 2.
    m1 coeff at lane p = cos_i * w[d].
    After the +-16 quadrant shuffle, lane p holds the partner dim value, so
    m2 coeff = -sin_i * w[d+1] for even d, +sin_i * w[d-1] for odd d."""
    n = cos.shape[0]
    cosT = np.empty((128, n), np.float32)
    sinT = np.empty((128, n), np.float32)
    for p in range(128):
        d = int(perm[p])
        i = d // 2
        cosT[p] = cos[:, i] * w[d]
        if d % 2 == 0:
            sinT[p] = -sin[:, i] * w[d + 1]
        else:
            sinT[p] = sin[:, i] * w[d - 1]
    return cosT, sinT


def _build_nc(debug=False):
    import concourse.bacc as bacc
    import concourse.mybir as mybir
    import concourse.tile as tile

    F32 = mybir.dt.float32
    F32R = mybir.dt.float32r
    AF = mybir.ActivationFunctionType
    SHUF_MASK = list(range(16, 32)) + list(range(0, 16))

    # Restrict ACT table-set choice to natural_log_exp_and_others (covers
    # Identity/Copy/Ln/Exp) so the whole kernel needs ONE table load instead
    # of alternating set loads (~1.3us each). Names/positions preserved so
    # act_func_set_id indices stay valid.
    _orig_tables = bacc.get_activation_tables

    def _one_set(arch):
        tabs = _orig_tables(arch)
        return {nm: (s if nm == "natural_log_exp_and_others" else set())
                for nm, s in tabs.items()}

    bacc.get_activation_tables = _one_set

    nc = bacc.Bacc("TRN2", target_bir_lowering=False, debug=False,
                   num_devices=N_CORES)

    # ---- DRAM I/O ----
    xT_d = nc.dram_tensor("xT", [C, N], F32R, kind="ExternalInput")
    wqk_d = nc.dram_tensor("wqkT", [C, 512], F32R, kind="ExternalInput")
    wv_d = nc.dram_tensor("wvT", [C, 256], F32R, kind="ExternalInput")
    pw_d = nc.dram_tensor("projwT", [256, C], F32R, kind="ExternalInput")
    bqk_d = nc.dram_tensor("bias_qk", [128, 4], F32, kind="ExternalInput")
    cq_d = nc.dram_tensor("cosq", [128, N], F32, kind="ExternalInput")
    sq_d = nc.dram_tensor("sinq", [128, N], F32, kind="ExternalInput")
    ck_d = nc.dram_tensor("cosk", [128, N], F32, kind="ExternalInput")
    sk_d = nc.dram_tensor("sink", [128, N], F32, kind="ExternalInput")
    ones_d = nc.dram_tensor("ones", [128, 1], F32R, kind="ExternalInput")
    eps_d = nc.dram_tensor("epsc", [1, 1], F32, kind="ExternalInput")
    out_d = nc.dram_tensor("partial", [N, C], F32, kind="ExternalOutput")
    if debug:
        dbg_qk = [nc.dram_tensor(f"dbg_qk{i}", [128, N], F32, kind="ExternalOutput")
                  for i in range(4)]
        dbg_v = nc.dram_tensor("dbg_v", [128, 16, 256], F32, kind="ExternalOutput")
        dbg_ctx = nc.dram_tensor("dbg_ctx", [128, 2, N], F32, kind="ExternalOutput")

    with tile.TileContext(nc) as tc:
        with (
            tc.tile_pool(name="persist", bufs=1) as pp,
            tc.tile_pool(name="rows", bufs=4) as rows,
            tc.tile_pool(name="tabp", bufs=1) as tabp,
        ):
            # resident SBUF tensors (per-chunk DMAs so compute can start
            # as soon as the first chunks land)
            wqk_sb = pp.tile([128, 16, 512], F32R, name="wqk_sb")
            wv_sb = pp.tile([128, 16, 256], F32R, name="wv_sb")
            pw_sb = pp.tile([128, 2, C], F32R, name="pw_sb")
            tab_dram = {"cq": cq_d, "sq": sq_d, "ck": ck_d, "sk": sk_d}
            bqk_sb = pp.tile([128, 4], F32, name="bqk_sb")
            nc.sync.dma_start(bqk_sb[:], bqk_d[:])
            ones_sb = pp.tile([128, 1], F32R, name="ones_sb")
            nc.sync.dma_start(ones_sb[:], ones_d[:])
            eps_sb = pp.tile([1, 1], F32, name="eps_sb")
            nc.sync.dma_start(eps_sb[:], eps_d[:])

            # final q/k (transposed, rope'd, scaled) and v, ctx
            qk_f = [pp.tile([128, N], F32R, name=f"qkf{i}") for i in range(4)]
            v_sb = pp.tile([128, 16, 256], F32R, name="v_sb")
            ctx_sb = pp.tile([128, 2, N], F32R, name="ctx_sb")

            # table per tensor index: 0:q0 1:k0 2:q1 3:k1
            tab_of = [("cq", "sq"), ("ck", "sk"), ("cq", "sq"), ("ck", "sk")]

            # ---------------- phase 1: QKV + RMS + RoPE ----------------
            with (
                tc.tile_pool(name="xt", bufs=6) as xtp,
                tc.tile_pool(name="qraw", bufs=6) as qrawp,
                tc.tile_pool(name="scr", bufs=3) as scr,
                tc.tile_pool(name="rbc", bufs=5) as rbcp,
                tc.tile_pool(name="ps_qk", bufs=4, space="PSUM") as ps_qk,
                tc.tile_pool(name="ps_v", bufs=1, space="PSUM") as ps_v,
                tc.tile_pool(name="ps_sq", bufs=2, space="PSUM") as ps_sq,
            ):
                def rope_stage(c4):
                    tsl = slice(c4 * 512, (c4 + 1) * 512)
                    tabt = {}
                    for nm in ("cq", "sq", "ck", "sk"):
                        tabt[nm] = tabp.tile([128, 512], F32, tag=nm,
                                             name=f"tab{nm}_{c4}")
                        nc.sync.dma_start(tabt[nm][:], tab_dram[nm][:, tsl])
                    # pass A: RMS factors (keeps the ssq matmuls unblocked on
                    # the PE FIFO after only the 4 cheap sq multiplies)
                    rbcs = {}
                    for f in (1, 3, 0, 2):   # k tensors first
                        qraw = qraw_tiles[(c4, f)]
                        sq = scr.tile([128, 512], F32R, tag="sq", name=f"sq{c4}_{f}")
                        nc.vector.tensor_mul(sq[:], qraw[:], qraw[:])
                        ssq = ps_sq.tile([1, 512], F32, tag="ssq", name=f"ssq{c4}_{f}")
                        nc.tensor.matmul(ssq[:], ones_sb[:], sq[:], start=True,
                                         stop=True)
                        lnr = rows.tile([1, 512], F32, tag="row", name=f"lnr{c4}_{f}")
                        nc.scalar.activation(lnr[:], ssq[:], AF.Ln,
                                             scale=1.0 / 128.0, bias=eps_sb[0:1, 0:1])
                        rrow = rows.tile([1, 512], F32, tag="row", name=f"rrow{c4}_{f}")
                        # r = mean_sq^-1/2 * D^-1/4  (D^-1/2 split across q and k)
                        nc.scalar.activation(rrow[:], lnr[:], AF.Exp, scale=-0.5,
                                             bias=_log_quarter(nc, pp))
                        rbc = rbcp.tile([128, 512], F32, tag="rbc", name=f"rbc{c4}_{f}")
                        nc.gpsimd.partition_broadcast(rbc[:], rrow[:])
                        rbcs[f] = rbc
                    # pass B: rotation + scaling
                    for f in (1, 3, 0, 2):
                        qraw = qraw_tiles[(c4, f)]
                        cosT = tabt[tab_of[f][0]]
                        sinT = tabt[tab_of[f][1]]
                        m1 = scr.tile([128, 512], F32, tag="m1", name=f"m1_{c4}_{f}")
                        nc.vector.tensor_mul(m1[:], qraw[:], cosT[:])
                        sh = scr.tile([128, 512], F32, tag="sh", name=f"sh{c4}_{f}")
                        nc.vector.stream_shuffle(sh[:], qraw[:], SHUF_MASK)
                        nc.vector.tensor_mul(sh[:], sh[:], sinT[:])
                        nc.vector.tensor_add(m1[:], m1[:], sh[:])
                        nc.vector.tensor_mul(qk_f[f][:, tsl], m1[:], rbcs[f][:])

                qraw_tiles = {}
                for c4 in range(4):
                    tsl = slice(c4 * 512, (c4 + 1) * 512)
                    qk_ps = [ps_qk.tile([128, 512], F32, tag="qkps", name=f"qkps{c4}_{_f}") for _f in range(4)]
                    # [128,1024] = 2 banks, two 256-wide v regions per bank.
                    # Only the first region per bank passes start=True (clears
                    # the whole bank); the second region's first matmul relies
                    # on the cleared has_written bits to overwrite, which is
                    # safe because the PE executes matmuls strictly in program
                    # order.
                    v_ps = ps_v.tile([128, 1024], F32, tag="vps", name=f"vps{c4}")
                    for i in range(16):
                        if c4 == 0:
                            nc.sync.dma_start(wqk_sb[:, i, :],
                                              wqk_d[i * 128:(i + 1) * 128, :])
                            nc.sync.dma_start(wv_sb[:, i, :],
                                              wv_d[i * 128:(i + 1) * 128, :])
                        xt = xtp.tile([128, 512], F32R, tag="xt", name=f"xt{c4}_{i}")
                        nc.sync.dma_start(xt[:], xT_d[i * 128:(i + 1) * 128, tsl])
                        for f in range(4):
                            nc.tensor.matmul(qk_ps[f][:],
                                             wqk_sb[:, i, f * 128:(f + 1) * 128],
                                             xt[:], start=(i == 0), stop=(i == 15))
                        for j in range(4):
                            nc.tensor.matmul(v_ps[:, j * 256:(j + 1) * 256],
                                             xt[:, j * 128:(j + 1) * 128],
                                             wv_sb[:, i, :],
                                             start=(i == 0 and j % 2 == 0),
                                             stop=(i == 15),
                                             skip_group_check=True)
                    # drain v: [tok 128, 256] tiles -> v_sb[:, kt, :]
                    for j in range(4):
                        kt = c4 * 4 + j
                        nc.vector.tensor_copy(v_sb[:, kt, :],
                                              v_ps[:, j * 256:(j + 1) * 256])
                    # drain q/k with bias; rope for the PREVIOUS chunk (keeps
                    # the PE FIFO free of ops that wait on the ACT/DVE chain)
                    for f in range(4):
                        qraw = qrawp.tile([128, 512], F32, tag="qraw", name=f"qraw{c4}_{f}")
                        nc.scalar.activation(qraw[:], qk_ps[f][:], AF.Identity,
                                             bias=bqk_sb[:, f:f + 1], scale=1.0)
                        qraw_tiles[(c4, f)] = qraw
                    if c4 > 0:
                        rope_stage(c4 - 1)
                rope_stage(3)

            for fc in range(2):
                nc.sync.dma_start(pw_sb[:, fc, :],
                                  pw_d[fc * 128:(fc + 1) * 128, :])

            # ------------- phase 2+3: attention + fused projection -------------
            with (
                tc.tile_pool(name="ep", bufs=4) as ep,
                tc.tile_pool(name="invb", bufs=3) as invbp,
                tc.tile_pool(name="outp", bufs=8) as outp,
                tc.tile_pool(name="ps_st", bufs=3, space="PSUM") as ps_st,
                tc.tile_pool(name="ps_ctx", bufs=2, space="PSUM") as ps_ctx,
                tc.tile_pool(name="ps_ssum", bufs=1, space="PSUM") as ps_ssum,
                tc.tile_pool(name="ps_o", bufs=2, space="PSUM") as ps_o,
            ):
                def proj_stage(qc, last=False):
                    for mt in range(4 * qc, 4 * qc + 4):
                        msl = slice(mt * 128, (mt + 1) * 128)
                        for oc in range(4):
                            osl = slice(oc * 512, (oc + 1) * 512)
                            po = ps_o.tile([128, 512], F32, tag="po", name=f"po{mt}_{oc}")
                            nc.tensor.matmul(po[:], ctx_sb[:, 0, msl], pw_sb[:, 0, osl],
                                             start=True, stop=False)
                            nc.tensor.matmul(po[:], ctx_sb[:, 1, msl], pw_sb[:, 1, osl],
                                             start=False, stop=True)
                            ot = outp.tile([128, 512], F32, tag="ot", name=f"ot{mt}_{oc}")
                            if last and oc % 2 == 1:
                                nc.scalar.copy(ot[:], po[:])
                            else:
                                nc.vector.tensor_copy(ot[:], po[:])
                            nc.sync.dma_start(out_d[msl, osl], ot[:])

                for qc in range(4):
                    qsl = slice(qc * 512, (qc + 1) * 512)
                    for h in range(2):
                        qT = qk_f[2 * h]
                        kT = qk_f[2 * h + 1]
                        ctx_ps = ps_ctx.tile([128, 512], F32, tag="ctxps", name=f"ctxps{h}_{qc}")
                        ssum = ps_ssum.tile([1, 512], F32, tag="ssum", name=f"ssum{h}_{qc}")
                        for kt in range(16):
                            st = ps_st.tile([128, 512], F32, tag="st", name=f"st{h}_{qc}_{kt}")
                            nc.tensor.matmul(st[:], kT[:, kt * 128:(kt + 1) * 128],
                                             qT[:, qsl], start=True, stop=True)
                            e = ep.tile([128, 512], F32R, tag="e", name=f"e{h}_{qc}_{kt}")
                            nc.scalar.activation(e[:], st[:], AF.Exp)
                            nc.tensor.matmul(ssum[:], ones_sb[:], e[:],
                                             start=(kt == 0), stop=(kt == 15))
                            nc.tensor.matmul(ctx_ps[:],
                                             v_sb[:, kt, h * 128:(h + 1) * 128],
                                             e[:], start=(kt == 0), stop=(kt == 15))
                        ssc = rows.tile([1, 512], F32, tag="row", name=f"ssc{h}_{qc}")
                        nc.vector.tensor_copy(ssc[:], ssum[:])
                        scr2 = rows.tile([1, 512], F32, tag="row", name=f"scr{h}_{qc}")
                        inv = rows.tile([1, 512], F32, tag="row", name=f"inv{h}_{qc}")
                        nc.vector.reciprocal_approx_accurate(inv[:], ssc[:], scr2[:])
                        invb = invbp.tile([128, 512], F32, tag="invb", name=f"invb{h}_{qc}")
                        nc.gpsimd.partition_broadcast(invb[:], inv[:])
                        nc.vector.tensor_mul(ctx_sb[:, h, qsl], ctx_ps[:], invb[:])
                    if qc > 0:
                        proj_stage(qc - 1)
                proj_stage(3, last=True)

            if debug:
                for i in range(4):
                    nc.sync.dma_start(dbg_qk[i][:], qk_f[i][:].bitcast(F32))
                nc.sync.dma_start(dbg_v[:], v_sb[:].bitcast(F32))
                nc.sync.dma_start(dbg_ctx[:], ctx_sb[:].bitcast(F32))

    try:
        nc.compile()
    finally:
        bacc.get_activation_tables = _orig_tables
    return nc


_LOGQ = {}


def _log_quarter(nc, pp):
    """[1,1] SBUF const holding -0.25*ln(128) (attention-scale split)."""
    import concourse.mybir as mybir
    key = id(nc)
    if key not in _LOGQ:
        t = pp.tile([1, 1], mybir.dt.float32, name="logq")
        nc.vector.memset(t[:], float(-0.25 * np.log(128.0)))
        _LOGQ[key] = t
    return _LOGQ[key][0:1, 0:1]


def _host_prep(x, qkv_w, qkv_b, proj_w, proj_b, q_norm_w, k_norm_w, T, H, W):
    perm = _perm_quadrant()
    cos, sin = _rope_tables(T, H, W, D)
    cosq, sinq = _folded_tables(cos, sin, np.asarray(q_norm_w, np.float32), perm)
    cosk, sink = _folded_tables(cos, sin, np.asarray(k_norm_w, np.float32), perm)

    xT = _round_f32r(np.asarray(x, np.float32)[0].T)
    qkv_w = np.asarray(qkv_w, np.float32)
    qkv_b = np.asarray(qkv_b, np.float32)
    proj_w = np.asarray(proj_w, np.float32)

    shared = dict(xT=xT, cosq=cosq, sinq=sinq, cosk=cosk, sink=sink,
                  ones=np.ones((128, 1), np.float32),
                  epsc=np.full((1, 1), EPS, np.float32))
    in_maps = []
    for c in range(N_CORES):
        h0 = 2 * c
        wq = [qkv_w[(h0 + j) * D:(h0 + j + 1) * D][perm] for j in range(2)]
        wk = [qkv_w[C + (h0 + j) * D:C + (h0 + j + 1) * D][perm] for j in range(2)]
        bq = [qkv_b[(h0 + j) * D:(h0 + j + 1) * D][perm] for j in range(2)]
        bk = [qkv_b[C + (h0 + j) * D:C + (h0 + j + 1) * D][perm] for j in range(2)]
        wqkT = np.concatenate([wq[0], wk[0], wq[1], wk[1]], axis=0).T
        bias_qk = np.stack([bq[0], bk[0], bq[1], bk[1]], axis=1)
        wvT = qkv_w[2 * C + h0 * D:2 * C + (h0 + 2) * D].T
        projwT = proj_w[:, h0 * D:(h0 + 2) * D].T
        in_maps.append(dict(shared,
                            wqkT=_round_f32r(wqkT),
                            wvT=_round_f32r(wvT),
                            projwT=_round_f32r(projwT),
                            bias_qk=np.ascontiguousarray(bias_qk)))
    v_bias = qkv_b[2 * C:]
    bias_row = (np.asarray(proj_b, np.float32).astype(np.float64)
                + v_bias.astype(np.float64) @ proj_w.astype(np.float64).T)
    return in_maps, bias_row


def kernel(x, qkv_w, qkv_b, proj_w, proj_b, q_norm_w, k_norm_w,
           t_dim, h_dim, w_dim):
    from concourse import bass_utils

    T, H, W = int(t_dim), int(h_dim), int(w_dim)
    if "nc" not in _CACHE:
        _CACHE["nc"] = _build_nc()
    nc = _CACHE["nc"]

    in_maps, bias_row = _host_prep(x, qkv_w, qkv_b, proj_w, proj_b,
                                   q_norm_w, k_norm_w, T, H, W)
    res = bass_utils.run_bass_kernel_spmd(nc, in_maps,
                                          core_ids=list(range(N_CORES)))
    total = np.zeros((N, C), np.float64)
    for r in res.results:
        total += r["partial"]
    out = (total + bias_row[None, :]).astype(np.float32)[None]
    return out
